# revision 3
# baseline (speedup 1.0000x reference)
"""Trainium2 Bass kernel for nn_CrossAttention (B_=64, N=512, C=128, heads=4).

Strategy: data-parallel over the B_ axis across 8 NeuronCores (8 windows per
core); parameters + relative-position-bias table replicated to every core.

Per (window, head) on device, with everything laid out transposed so that no
on-device transposes are ever needed:
    qT = (Wq*scale) @ xT          (hd=32 rows per head, 512 cols)   [PE]
    kT = Wk @ yT                                                     [PE]
    v  = yT.T @ WvT               (natural layout, k-rows x (h,hd))  [PE]
    ST = kT.T@qT + R^T  per 128-row k-chunk into PSUM                [PE]
    P  = exp(ST)  straight out of PSUM (no softmax max-subtraction:
                  |logits| < ~0.7 at this problem scale)             [ACT]
    OT = v.T @ P, den = 1.T @ P   (column-packed matmuls, 4 heads concurrent)
    OTn = OT * bcast(1/den)       (den rows replicated per head by the ones
                                   lhsT, so 1/d_ps IS the broadcast divisor)
    outT = projwT.T @ OTn + pb    (transposed proj: one 512-col matmul, bias
                                   add on DVE; host un-transposes the output)

The R^T bias is injected into the QK PSUM accumulation as four K=32
"selector" matmuls per (head, chunk), each targeting a 32x32 sub-array the
QK matmuls of that head-pair don't use (QKs of the pair sit on array-row
groups r0/r1, the sub-adds on r2/r3 and vice versa), so the whole chunk --
2 QKs + 8 bias adds -- streams concurrently through disjoint sub-arrays.
This keeps the softmax bias entirely off the Vector engine and leaves the
Scalar engine's exp (1 elem/cycle/lane) as the only elementwise floor.
"""

import sys

sys.path.insert(0, "/opt/trn_rl_repo")

import numpy as np
import ml_dtypes

from contextlib import ExitStack

import concourse.bass as bass
import concourse.tile as tile
from concourse import bacc, mybir
from concourse import bass_utils

FP32 = mybir.dt.float32
BF16 = mybir.dt.bfloat16

# problem constants (hardcoded per spec: x,y are (64, 512, 128), H=W=D=8)
B_, N, C, HEADS, HD = 64, 512, 128, 4, 32
NCORES = 8
WIN = B_ // NCORES  # windows per core
POS_DIM = 8
KC = N // 128  # 4 k-chunks of 128
# array-row group used by head h's bias sub-adds (disjoint from the row
# groups its pair's QK matmuls occupy); also the partition base of head h's
# slice of the bias table
RADD_ROW = {0: 2, 1: 3, 2: 0, 3: 1}


def _layernorm(x, g, b, eps=1e-5):
    m = x.mean(-1, keepdims=True)
    v = x.var(-1, keepdims=True)
    return (x - m) / np.sqrt(v + eps) * g + b


def _rel_pos_tables(H, W, D):
    bh = np.arange(1 - H, H)
    bw = np.arange(1 - W, W)
    bd = np.arange(1 - D, D)
    biases = np.stack(np.meshgrid(bh, bw, bd, indexing="ij")).reshape(3, -1).T
    coords = np.stack(
        np.meshgrid(np.arange(H), np.arange(W), np.arange(D), indexing="ij")
    ).reshape(3, -1)
    rel = coords[:, :, None] - coords[:, None, :]
    rel = rel.transpose(1, 2, 0).astype(np.int64)
    rel[:, :, 0] += H - 1
    rel[:, :, 1] += W - 1
    rel[:, :, 2] += D - 1
    rel[:, :, 0] *= (2 * W - 1) * (2 * D - 1)
    rel[:, :, 1] *= 2 * D - 1
    idx = rel.sum(-1)
    return biases.astype(np.float32), idx


def _build_program():
    """Build the Bass/Tile program once; returns the Bass object."""
    nc = bacc.Bacc("TRN2", target_bir_lowering=False, debug=False)

    # per-core inputs
    xT_d = nc.dram_tensor("xT", (WIN, C, N), BF16, kind="ExternalInput")
    yT_d = nc.dram_tensor("yT", (WIN, C, N), BF16, kind="ExternalInput")
    # R^T quarters keyed for the sub-add matmuls:
    # rT4[32*RADD_ROW[h]+p, (kc*4+i)*512+q] = R_h[q, kc*128+32*i+p]
    rT4_d = nc.dram_tensor("rT4", (128, KC * 4 * N), BF16, kind="ExternalInput")
    eye_d = nc.dram_tensor("eye32", (128, 32), BF16, kind="ExternalInput")
    wq_d = nc.dram_tensor("wqT", (C, C), BF16, kind="ExternalInput")
    wk_d = nc.dram_tensor("wkT", (C, C), BF16, kind="ExternalInput")
    wv_d = nc.dram_tensor("wvT", (C, C), BF16, kind="ExternalInput")
    pw_d = nc.dram_tensor("projwT", (C, C), BF16, kind="ExternalInput")
    # proj bias in [c, q] layout: per-partition constant, replicated along q
    pb_d = nc.dram_tensor("pbCN", (128, N), FP32, kind="ExternalInput")
    # output is stored transposed per window: (C, N); host un-transposes
    out_d = nc.dram_tensor("outT", (WIN, C, N), FP32, kind="ExternalOutput")

    with TileCtx(nc) as tc, ExitStack() as ctx:
        const = ctx.enter_context(tc.tile_pool(name="const", bufs=1))
        xy = ctx.enter_context(tc.tile_pool(name="xy", bufs=4))
        qk_sb = ctx.enter_context(tc.tile_pool(name="qk_sb", bufs=4))
        v_pool = ctx.enter_context(tc.tile_pool(name="v_sb", bufs=3))
        p_pool = ctx.enter_context(tc.tile_pool(name="p_sb", bufs=6))
        misc = ctx.enter_context(tc.tile_pool(name="misc", bufs=2))
        outp = ctx.enter_context(tc.tile_pool(name="out_sb", bufs=2))
        mm_ps = ctx.enter_context(
            tc.tile_pool(name="mm_ps", bufs=2, space=bass.MemorySpace.PSUM)
        )
        st_ps = ctx.enter_context(
            tc.tile_pool(name="st_ps", bufs=2, space=bass.MemorySpace.PSUM)
        )

        # ---- constants, loaded once ----
        wq_sb = const.tile([C, C], BF16, tag="wq")
        wk_sb = const.tile([C, C], BF16, tag="wk")
        wv_sb = const.tile([C, C], BF16, tag="wv")
        pw_sb = const.tile([C, C], BF16, tag="pw")
        pb_sb = const.tile([128, N], FP32, tag="pb")
        eye_sb = const.tile([128, 32], BF16, tag="eye")
        rT4_sb = const.tile([128, KC * 4 * N], BF16, tag="rT4")
        ones_sb = const.tile([128, 32], BF16, tag="ones")
        zeros_sb = const.tile([128, 128], BF16, tag="zeros")
        for dst, src in (
            (wq_sb, wq_d), (wk_sb, wk_d), (wv_sb, wv_d), (pw_sb, pw_d),
            (eye_sb, eye_d),
        ):
            nc.gpsimd.dma_start(dst[:], src[:])
        nc.gpsimd.dma_start(pb_sb[:], pb_d[:])
        # the 2MB bias table rides the SWDGE ring so window-0 x/y loads
        # (HWDGE) are not queued behind it
        for i in range(4):
            nc.gpsimd.dma_start(
                rT4_sb[:, i * KC * N : (i + 1) * KC * N],
                rT4_d[:, i * KC * N : (i + 1) * KC * N],
            )
        nc.vector.memset(ones_sb[:], 1.0)
        nc.vector.memset(zeros_sb[:], 0.0)

        # ---- per-window pipeline ----
        for b in range(WIN):
            xt = xy.tile([C, N], BF16, tag="xt")
            yt = xy.tile([C, N], BF16, tag="yt")
            nc.sync.dma_start(xt[:], xT_d[b])
            nc.sync.dma_start(yt[:], yT_d[b])

            qT_ps = mm_ps.tile([128, N], FP32, tag="ps1")
            kT_ps = mm_ps.tile([128, N], FP32, tag="ps1")
            v_ps = mm_ps.tile([128, N], FP32, tag="ps1")
            nc.tensor.matmul(qT_ps[:], lhsT=wq_sb[:], rhs=xt[:], start=True, stop=True)
            nc.tensor.matmul(kT_ps[:], lhsT=wk_sb[:], rhs=yt[:], start=True, stop=True)
            for j in range(4):
                nc.tensor.matmul(
                    v_ps[:, j * 128 : (j + 1) * 128],
                    lhsT=yt[:, j * 128 : (j + 1) * 128],
                    rhs=wv_sb[:],
                    start=True,
                    stop=True,
                    skip_group_check=True,
                )
            qT_sb = qk_sb.tile([128, N], BF16, tag="qT")
            kT_sb = qk_sb.tile([128, N], BF16, tag="kT")
            v_sb = v_pool.tile([128, N], BF16, tag="v")
            nc.vector.tensor_copy(qT_sb[:], qT_ps[:])
            nc.vector.tensor_copy(kT_sb[:], kT_ps[:])
            nc.vector.tensor_copy(v_sb[:], v_ps[:])

            # S^T tiles + exp, per (head-pair, half): unit is (128, 1024)
            # = 2 k-chunks for one head. Within a chunk the pair's two QK
            # matmuls (row groups 32h) and the eight bias sub-adds (the
            # other two row groups x all four col groups) are disjoint on
            # the PE array, so all ten stream concurrently.
            p_tiles = {}
            p_prods = {}
            for hf in range(2):
                for pair in ((0, 1), (2, 3)):
                    sts = [
                        st_ps.tile([128, 1024], FP32, tag="st", name=f"st{i}")
                        for i in range(2)
                    ]
                    for j in range(2):
                        kc = 2 * hf + j
                        sl = slice(j * 512, (j + 1) * 512)
                        for t, h in enumerate(pair):
                            nc.tensor.matmul(
                                sts[t][:, sl],
                                lhsT=kT_sb[32 * h : 32 * h + 32, kc * 128 : (kc + 1) * 128],
                                rhs=qT_sb[32 * h : 32 * h + 32, :],
                                start=True,
                                stop=False,
                                tile_position=(32 * h, 0),
                                skip_group_check=True,
                            )
                        for t, h in enumerate(pair):
                            rb = 32 * RADD_ROW[h]
                            for i in range(4):
                                nc.tensor.matmul(
                                    sts[t][32 * i : 32 * i + 32, sl],
                                    lhsT=eye_sb[rb : rb + 32, :],
                                    rhs=rT4_sb[rb : rb + 32, (kc * 4 + i) * N : (kc * 4 + i + 1) * N],
                                    start=False,
                                    stop=(i == 3),
                                    tile_position=(rb, 32 * i),
                                    skip_group_check=True,
                                )
                    for t, h in enumerate(pair):
                        p = p_pool.tile([128, 1024], BF16, tag="p")
                        ei = nc.scalar.activation(
                            p[:], sts[t][:], mybir.ActivationFunctionType.Exp
                        )
                        p_tiles[(h, hf)] = p
                        p_prods[(h, hf)] = ei.ins

            # O^T (col-packed, 4 heads) + denominators
            ot_ps = mm_ps.tile([128, N], FP32, tag="ps2")
            d_ps = mm_ps.tile([128, N], FP32, tag="ps2")
            # Open each accumulation bank with a zeroing matmul: clears
            # has_written for the whole bank AND writes zeros to all 128
            # partitions, so the per-head chains below can all accumulate
            # with start=False (correct under both per-element-sim and
            # whole-bank-HW has_written semantics). K=32 on distinct row
            # groups so the two openers stream concurrently and stay off
            # the sub-arrays other matmuls need.
            nc.tensor.matmul(
                ot_ps[:], lhsT=zeros_sb[0:32, :], rhs=rT4_sb[0:32, 0:N],
                start=True, stop=False, tile_position=(0, 0),
                skip_group_check=True,
            )
            nc.tensor.matmul(
                d_ps[:], lhsT=zeros_sb[32:64, :], rhs=rT4_sb[32:64, 0:N],
                start=True, stop=False, tile_position=(32, 0),
                skip_group_check=True,
            )
            # Each 4-head group sits behind no-sync hints on all 4 P
            # producers so the four col-strip matmuls stay adjacent on PE
            # and run concurrently.
            for kc in range(KC):
                hf = kc // 2
                group_deps = [p_prods[(h, hf)] for h in range(HEADS)]
                for h in range(HEADS):
                    p = p_tiles[(h, hf)]
                    psl = p[:, (kc % 2) * 512 : (kc % 2 + 1) * 512]
                    mm1 = nc.tensor.matmul(
                        ot_ps[32 * h : 32 * h + 32, :],
                        lhsT=v_sb[:, kc * 128 + 32 * h : kc * 128 + 32 * h + 32],
                        rhs=psl,
                        start=False,
                        stop=(kc == KC - 1),
                        tile_position=(0, 32 * h),
                        skip_group_check=True,
                    )
                    mm2 = nc.tensor.matmul(
                        d_ps[32 * h : 32 * h + 32, :],
                        lhsT=ones_sb[:],
                        rhs=psl,
                        start=False,
                        stop=(kc == KC - 1),
                        tile_position=(0, 32 * h),
                        skip_group_check=True,
                    )
                    for d in group_deps:
                        tile.add_dep_helper(mm1.ins, d, False, "pv pack")
                        tile.add_dep_helper(mm2.ins, d, False, "pv pack")

            # d_ps rows 32h..32h+31 all hold head h's denominator (the ones
            # lhsT replicates it), so 1/d_ps IS the broadcast divisor.
            # 18-bit approx is plenty: den ~ 512 +- 15%.
            invden = misc.tile([128, N], FP32, tag="invden")
            nc.vector.reciprocal_approx_fast(invden[:], d_ps[:])
            otn = misc.tile([128, N], BF16, tag="otn")
            nc.vector.tensor_mul(otn[:], ot_ps[:], invden[:])

            # transposed proj: outT[c, q] = sum_hd projwT[hd, c] * otn[hd, q]
            pr_ps = mm_ps.tile([128, N], FP32, tag="ps2")
            nc.tensor.matmul(pr_ps[:], lhsT=pw_sb[:], rhs=otn[:], start=True, stop=True)
            ot = outp.tile([128, N], FP32, tag="out")
            nc.vector.tensor_add(ot[:], pr_ps[:], pb_sb[:])
            nc.sync.dma_start(out_d[b], ot[:])
    nc.compile()
    return nc


def TileCtx(nc):
    return tile.TileContext(nc)


_CACHE = {}


def _get_program():
    if "nc" not in _CACHE:
        _CACHE["nc"] = _build_program()
    return _CACHE["nc"]


def _host_prep(x, y, H, W, D, qkv_w, qkv_b, proj_w, proj_b,
               pos_proj_w, pos_proj_b, ln1_g, ln1_b, p1_w, p1_b,
               ln2_g, ln2_b, p2_w, p2_b, ln3_g, ln3_b, p3_w, p3_b):
    """Numpy-only prep: layout transforms, weight folding, pos-bias table."""
    scale = HD ** -0.5
    bf = ml_dtypes.bfloat16

    xT = np.ascontiguousarray(x.transpose(0, 2, 1)).astype(bf)  # (B_, C, N)
    yT = np.ascontiguousarray(y.transpose(0, 2, 1)).astype(bf)

    wqT = np.ascontiguousarray((qkv_w[0:C] * scale).T).astype(bf)
    wkT = np.ascontiguousarray(qkv_w[C : 2 * C].T).astype(bf)
    wvT = np.ascontiguousarray(qkv_w[2 * C : 3 * C].T).astype(bf)
    projwT = np.ascontiguousarray(proj_w.T).astype(bf)

    # pos-bias MLP (tiny: 3375x8), exact fp32 replica of the reference math
    biases, idx = _rel_pos_tables(int(H), int(W), int(D))
    pos = biases @ pos_proj_w.T + pos_proj_b
    pos = np.maximum(_layernorm(pos, ln1_g, ln1_b), 0) @ p1_w.T + p1_b
    pos = np.maximum(_layernorm(pos, ln2_g, ln2_b), 0) @ p2_w.T + p2_b
    pos = np.maximum(_layernorm(pos, ln3_g, ln3_b), 0) @ p3_w.T + p3_b  # (T, h)
    rpb = pos[idx.reshape(-1)].reshape(N, N, HEADS)  # [q, k, h]
    bq = qkv_b[0:C]
    bk = qkv_b[C : 2 * C]
    if np.any(bq) or np.any(bk):
        raise NotImplementedError("nonzero qkv bias not supported")
    # sub-add table: rT4[32*RADD_ROW[h]+p, (kc*4+i)*512+q] = R_h[q, kc*128+32i+p]
    rT4 = np.zeros((128, KC * 4 * N), np.float32)
    for h in range(HEADS):
        rb = 32 * RADD_ROW[h]
        RhT = rpb[:, :, h].T  # [k, q]
        # [KC*4 quarters, 32, q] -> partition p, col (quarter*512+q)
        quarters = RhT.reshape(KC * 4, 32, N)
        rT4[rb : rb + 32, :] = quarters.transpose(1, 0, 2).reshape(32, KC * 4 * N)
    rT4 = rT4.astype(bf)

    eye32 = np.tile(np.eye(32, dtype=np.float32), (4, 1)).astype(bf)

    pb_full = proj_b + qkv_b[2 * C : 3 * C] @ proj_w.T  # fold v bias thru proj
    pbCN = np.tile(pb_full[:, None], (1, N)).astype(np.float32)  # (C, N)

    return xT, yT, rT4, eye32, wqT, wkT, wvT, projwT, pbCN


def kernel(**inputs):
    inputs = {k: np.asarray(v) if not np.isscalar(v) else v for k, v in inputs.items()}
    x = np.asarray(inputs["x"], np.float32)
    assert x.shape == (B_, N, C)
    xT, yT, rT4, eye32, wqT, wkT, wvT, projwT, pbCN = _host_prep(
        np.asarray(inputs["x"], np.float32),
        np.asarray(inputs["y"], np.float32),
        inputs["H"], inputs["W"], inputs["D"],
        np.asarray(inputs["qkv_w"], np.float32),
        np.asarray(inputs["qkv_b"], np.float32),
        np.asarray(inputs["proj_w"], np.float32),
        np.asarray(inputs["proj_b"], np.float32),
        np.asarray(inputs["pos_proj_w"], np.float32),
        np.asarray(inputs["pos_proj_b"], np.float32),
        np.asarray(inputs["ln1_g"], np.float32), np.asarray(inputs["ln1_b"], np.float32),
        np.asarray(inputs["p1_w"], np.float32), np.asarray(inputs["p1_b"], np.float32),
        np.asarray(inputs["ln2_g"], np.float32), np.asarray(inputs["ln2_b"], np.float32),
        np.asarray(inputs["p2_w"], np.float32), np.asarray(inputs["p2_b"], np.float32),
        np.asarray(inputs["ln3_g"], np.float32), np.asarray(inputs["ln3_b"], np.float32),
        np.asarray(inputs["p3_w"], np.float32), np.asarray(inputs["p3_b"], np.float32),
    )

    nc = _get_program()
    in_maps = []
    for c in range(NCORES):
        sl = slice(c * WIN, (c + 1) * WIN)
        in_maps.append(
            {
                "xT": xT[sl],
                "yT": yT[sl],
                "rT4": rT4,
                "eye32": eye32,
                "wqT": wqT,
                "wkT": wkT,
                "wvT": wvT,
                "projwT": projwT,
                "pbCN": pbCN,
            }
        )
    kwargs = {}
    if PROFILE:
        kwargs = dict(trace=True, **PROFILE_KWARGS)
    res = bass_utils.run_bass_kernel_spmd(
        nc, in_maps, core_ids=list(range(NCORES)), **kwargs
    )
    global LAST_EXEC_NS, LAST_RESULTS
    LAST_EXEC_NS = res.exec_time_ns
    LAST_RESULTS = res
    # outT is (WIN, C, N); un-transpose to (WIN, N, C) on host
    out = np.concatenate(
        [np.asarray(r["outT"]).transpose(0, 2, 1) for r in res.results], axis=0
    )
    return np.ascontiguousarray(out).astype(np.float32)


PROFILE = False
PROFILE_KWARGS = {}
LAST_EXEC_NS = None
LAST_RESULTS = None


if __name__ == "__main__":
    # smoke test with random data
    rng = np.random.default_rng(0)
    demo = {
        "x": rng.standard_normal((B_, N, C)).astype(np.float32),
        "y": rng.standard_normal((B_, N, C)).astype(np.float32),
        "H": 8, "W": 8, "D": 8,
        "qkv_w": (rng.standard_normal((3 * C, C)) * 0.02).astype(np.float32),
        "qkv_b": np.zeros(3 * C, np.float32),
        "proj_w": (rng.standard_normal((C, C)) * 0.02).astype(np.float32),
        "proj_b": np.zeros(C, np.float32),
        "pos_proj_w": (rng.standard_normal((POS_DIM, 3)) * 0.02).astype(np.float32),
        "pos_proj_b": np.zeros(POS_DIM, np.float32),
        "ln1_g": np.ones(POS_DIM, np.float32), "ln1_b": np.zeros(POS_DIM, np.float32),
        "p1_w": (rng.standard_normal((POS_DIM, POS_DIM)) * 0.02).astype(np.float32),
        "p1_b": np.zeros(POS_DIM, np.float32),
        "ln2_g": np.ones(POS_DIM, np.float32), "ln2_b": np.zeros(POS_DIM, np.float32),
        "p2_w": (rng.standard_normal((POS_DIM, POS_DIM)) * 0.02).astype(np.float32),
        "p2_b": np.zeros(POS_DIM, np.float32),
        "ln3_g": np.ones(POS_DIM, np.float32), "ln3_b": np.zeros(POS_DIM, np.float32),
        "p3_w": (rng.standard_normal((HEADS, POS_DIM)) * 0.02).astype(np.float32),
        "p3_b": np.zeros(HEADS, np.float32),
    }
    out = kernel(**demo)
    print("kernel out:", out.shape, out.dtype, np.abs(out).max())


# revision 13
# speedup vs baseline: 1.6577x; 1.6577x over previous
"""Trainium2 Bass kernel for nn_CrossAttention (B_=64, N=512, C=128, heads=4).

Strategy: data-parallel over the B_ axis across 8 NeuronCores (8 windows per
core); parameters + relative-position-bias tables replicated to every core.

Per (window, head) on device, with everything laid out transposed so that no
on-device transposes are ever needed:
    qT = (Wq*scale) @ xT          (hd=32 rows per head, 512 cols)   [PE]
    kT = Wk @ yT                                                     [PE]
    v  = yT.T @ WvT               (natural layout, k-rows x (h,hd))  [PE]
    ST = kT.T@qT  per 128-row k-chunk into PSUM                      [PE]
    heads 0,1:  ST += R^T via identity matmul (PSUM accumulate), then
                P = exp(ST) straight out of PSUM                     [ACT]
    heads 2,3:  P = exp(ST) * exp(R)^T  (exp on ACT, mul on DVE)
    (no softmax max-subtraction: |logits| < ~0.7 at this problem scale)
    OT = v.T @ P, den = 1.T @ P   (column-packed matmuls, 4 heads concurrent)
    OTn = OT * bcast(1/den)       (den rows replicated per head by the ones
                                   lhsT, so 1/d_ps IS the broadcast divisor)
    outT = projwT.T @ OTn         (transposed proj: one 512-col matmul,
                                   DMA'd straight from PSUM; the proj bias
                                   add and the un-transpose happen on host)

The softmax bias is split between PE (identity-matmul adds for two heads)
and DVE (exp(R) multiplies for the other two) so neither engine becomes the
bottleneck; ACT's exp at 1 elem/cycle/lane is the irreducible floor. Within
an S-group the QK matmuls are issued adjacently (they pack on distinct PE
row groups) before the full-array bias adds, which cannot pack.
"""

import sys

sys.path.insert(0, "/opt/trn_rl_repo")

import numpy as np
import ml_dtypes

from contextlib import ExitStack

import concourse.bass as bass
import concourse.tile as tile
from concourse import bacc, mybir
from concourse import bass_utils

FP32 = mybir.dt.float32
BF16 = mybir.dt.bfloat16

# problem constants (hardcoded per spec: x,y are (64, 512, 128), H=W=D=8)
B_, N, C, HEADS, HD = 64, 512, 128, 4, 32
NCORES = 8
WIN = B_ // NCORES  # windows per core
POS_DIM = 8
KC = N // 128  # 4 k-chunks of 128
PE_HEADS = (0, 1)   # bias added on PE (identity matmul into PSUM)
DVE_HEADS = (2, 3)  # bias multiplied on DVE (exp(S) * exp(R))


def _layernorm(x, g, b, eps=1e-5):
    m = x.mean(-1, keepdims=True)
    v = x.var(-1, keepdims=True)
    return (x - m) / np.sqrt(v + eps) * g + b


def _rel_pos_tables(H, W, D):
    bh = np.arange(1 - H, H)
    bw = np.arange(1 - W, W)
    bd = np.arange(1 - D, D)
    biases = np.stack(np.meshgrid(bh, bw, bd, indexing="ij")).reshape(3, -1).T
    coords = np.stack(
        np.meshgrid(np.arange(H), np.arange(W), np.arange(D), indexing="ij")
    ).reshape(3, -1)
    rel = coords[:, :, None] - coords[:, None, :]
    rel = rel.transpose(1, 2, 0).astype(np.int64)
    rel[:, :, 0] += H - 1
    rel[:, :, 1] += W - 1
    rel[:, :, 2] += D - 1
    rel[:, :, 0] *= (2 * W - 1) * (2 * D - 1)
    rel[:, :, 1] *= 2 * D - 1
    idx = rel.sum(-1)
    return biases.astype(np.float32), idx


def _build_program():
    """Build the Bass/Tile program once; returns the Bass object."""
    nc = bacc.Bacc("TRN2", target_bir_lowering=False, debug=False)

    # per-core inputs
    xT_d = nc.dram_tensor("xT", (WIN, C, N), BF16, kind="ExternalInput")
    yT_d = nc.dram_tensor("yT", (WIN, C, N), BF16, kind="ExternalInput")
    # R^T for PE-path heads: col (h'*KC+kc)*N+q, partition p = k within chunk
    rT_d = nc.dram_tensor("rT", (128, len(PE_HEADS) * KC * N), BF16, kind="ExternalInput")
    # exp(R)^T for DVE-path heads, same layout
    erT_d = nc.dram_tensor("expRT", (128, len(DVE_HEADS) * KC * N), BF16, kind="ExternalInput")
    id_d = nc.dram_tensor("ident", (128, 128), BF16, kind="ExternalInput")
    wq_d = nc.dram_tensor("wqT", (C, C), BF16, kind="ExternalInput")
    wk_d = nc.dram_tensor("wkT", (C, C), BF16, kind="ExternalInput")
    wv_d = nc.dram_tensor("wvT", (C, C), BF16, kind="ExternalInput")
    pw_d = nc.dram_tensor("projwT", (C, C), BF16, kind="ExternalInput")
    # proj bias in [c, q] layout: per-partition constant, replicated along q
    pb_d = nc.dram_tensor("pbCN", (128, N), FP32, kind="ExternalInput")
    # output is stored transposed per window: (C, N); host un-transposes
    out_d = nc.dram_tensor("outT", (WIN, C, N), FP32, kind="ExternalOutput")

    with TileCtx(nc) as tc, ExitStack() as ctx:
        const = ctx.enter_context(tc.tile_pool(name="const", bufs=1))
        xy = ctx.enter_context(tc.tile_pool(name="xy", bufs=4))
        qk_sb = ctx.enter_context(tc.tile_pool(name="qk_sb", bufs=4))
        v_pool = ctx.enter_context(tc.tile_pool(name="v_sb", bufs=3))
        p_pool = ctx.enter_context(tc.tile_pool(name="p_sb", bufs=6))
        misc = ctx.enter_context(tc.tile_pool(name="misc", bufs=2))
        outp = ctx.enter_context(tc.tile_pool(name="out_sb", bufs=2))
        mm_ps = ctx.enter_context(
            tc.tile_pool(name="mm_ps", bufs=2, space=bass.MemorySpace.PSUM)
        )
        st_ps = ctx.enter_context(
            tc.tile_pool(name="st_ps", bufs=2, space=bass.MemorySpace.PSUM)
        )

        # ---- constants, loaded once ----
        wq_sb = const.tile([C, C], BF16, tag="wq")
        wk_sb = const.tile([C, C], BF16, tag="wk")
        wv_sb = const.tile([C, C], BF16, tag="wv")
        pw_sb = const.tile([C, C], BF16, tag="pw")
        id_sb = const.tile([128, 128], BF16, tag="ident")
        pb_sb = const.tile([128, N], FP32, tag="pb")
        rT_sb = const.tile([128, len(PE_HEADS) * KC * N], BF16, tag="rT")
        erT_sb = const.tile([128, len(DVE_HEADS) * KC * N], BF16, tag="erT")
        ones_sb = const.tile([128, 32], BF16, tag="ones")
        zeros_sb = const.tile([128, 128], BF16, tag="zeros")
        for dst, src in (
            (wq_sb, wq_d), (wk_sb, wk_d), (wv_sb, wv_d), (pw_sb, pw_d),
            (id_sb, id_d),
        ):
            nc.gpsimd.dma_start(dst[:], src[:])
        nc.gpsimd.dma_start(pb_sb[:], pb_d[:])
        # bias tables ride the SWDGE ring so window-0 x/y loads (HWDGE)
        # are not queued behind them
        for i in range(len(PE_HEADS)):
            nc.gpsimd.dma_start(
                rT_sb[:, i * KC * N : (i + 1) * KC * N],
                rT_d[:, i * KC * N : (i + 1) * KC * N],
            )
        for i in range(len(DVE_HEADS)):
            nc.gpsimd.dma_start(
                erT_sb[:, i * KC * N : (i + 1) * KC * N],
                erT_d[:, i * KC * N : (i + 1) * KC * N],
            )
        nc.vector.memset(ones_sb[:], 1.0)
        nc.vector.memset(zeros_sb[:], 0.0)

        # ---- per-window pipeline ----
        for b in range(WIN):
            xt = xy.tile([C, N], BF16, tag="xt")
            yt = xy.tile([C, N], BF16, tag="yt")
            nc.sync.dma_start(xt[:], xT_d[b])
            nc.sync.dma_start(yt[:], yT_d[b])

            qT_ps = mm_ps.tile([128, N], FP32, tag="ps1")
            kT_ps = mm_ps.tile([128, N], FP32, tag="ps1")
            v_ps = mm_ps.tile([128, N], FP32, tag="ps1")
            nc.tensor.matmul(qT_ps[:], lhsT=wq_sb[:], rhs=xt[:], start=True, stop=True)
            nc.tensor.matmul(kT_ps[:], lhsT=wk_sb[:], rhs=yt[:], start=True, stop=True)
            for j in range(4):
                nc.tensor.matmul(
                    v_ps[:, j * 128 : (j + 1) * 128],
                    lhsT=yt[:, j * 128 : (j + 1) * 128],
                    rhs=wv_sb[:],
                    start=True,
                    stop=True,
                    skip_group_check=True,
                )
            qT_sb = qk_sb.tile([128, N], BF16, tag="qT")
            kT_sb = qk_sb.tile([128, N], BF16, tag="kT")
            v_sb = v_pool.tile([128, N], BF16, tag="v")
            nc.vector.tensor_copy(qT_sb[:], qT_ps[:])
            nc.vector.tensor_copy(kT_sb[:], kT_ps[:])
            nc.vector.tensor_copy(v_sb[:], v_ps[:])

            # O^T / den accumulator banks, opened early with zeroing matmuls:
            # clears has_written for the whole bank AND writes zeros to all
            # 128 partitions, so the per-head chains below can all accumulate
            # with start=False (correct under both per-element-sim and
            # whole-bank-HW has_written semantics). K=32 on row groups 2/3,
            # which the first S-group's QK matmuls (rows 0/1) don't use.
            ot_ps = mm_ps.tile([128, N], FP32, tag="ps2")
            d_ps = mm_ps.tile([128, N], FP32, tag="ps2")
            nc.tensor.matmul(
                ot_ps[:], lhsT=zeros_sb[64:96, :], rhs=rT_sb[64:96, 0:N],
                start=True, stop=False, tile_position=(64, 0),
                skip_group_check=True,
            )
            nc.tensor.matmul(
                d_ps[:], lhsT=zeros_sb[96:128, :], rhs=rT_sb[96:128, 0:N],
                start=True, stop=False, tile_position=(96, 0),
                skip_group_check=True,
            )

            # S^T tiles + exp, per (head-pair, half): unit is (128, 1024)
            # = 2 k-chunks for one head. The pair's four QK matmuls are
            # issued adjacently so they stream concurrently on distinct PE
            # row groups; the full-array bias adds follow.
            p_tiles = {}
            p_prods = {}
            for hf in range(2):
                for pair in (PE_HEADS, DVE_HEADS):
                    on_pe = pair is PE_HEADS
                    sts = [
                        st_ps.tile([128, 1024], FP32, tag="st", name=f"st{i}")
                        for i in range(2)
                    ]
                    for j in range(2):
                        kc = 2 * hf + j
                        sl = slice(j * 512, (j + 1) * 512)
                        for t, h in enumerate(pair):
                            nc.tensor.matmul(
                                sts[t][:, sl],
                                lhsT=kT_sb[32 * h : 32 * h + 32, kc * 128 : (kc + 1) * 128],
                                rhs=qT_sb[32 * h : 32 * h + 32, :],
                                start=True,
                                stop=not on_pe,
                                tile_position=(32 * h, 0),
                                skip_group_check=True,
                            )
                    if on_pe:
                        for j in range(2):
                            kc = 2 * hf + j
                            sl = slice(j * 512, (j + 1) * 512)
                            for t, h in enumerate(pair):
                                hi = PE_HEADS.index(h)
                                nc.tensor.matmul(
                                    sts[t][:, sl],
                                    lhsT=id_sb[:],
                                    rhs=rT_sb[:, (hi * KC + kc) * N : (hi * KC + kc + 1) * N],
                                    start=False,
                                    stop=True,
                                    skip_group_check=True,
                                )
                    for t, h in enumerate(pair):
                        if on_pe:
                            p = p_pool.tile([128, 1024], BF16, tag="p01")
                            ei = nc.scalar.activation(
                                p[:], sts[t][:], mybir.ActivationFunctionType.Exp
                            )
                            p_tiles[(h, hf)] = p
                            p_prods[(h, hf)] = ei.ins
                        else:
                            praw = p_pool.tile([128, 1024], BF16, tag="praw")
                            nc.scalar.activation(
                                praw[:], sts[t][:], mybir.ActivationFunctionType.Exp
                            )
                            hi = DVE_HEADS.index(h)
                            p = p_pool.tile([128, 1024], BF16, tag="p23")
                            mi = nc.vector.tensor_mul(
                                p[:],
                                praw[:],
                                erT_sb[:, (hi * KC + 2 * hf) * N : (hi * KC + 2 * hf + 2) * N],
                            )
                            p_tiles[(h, hf)] = p
                            p_prods[(h, hf)] = mi.ins

            # Each 4-head O^T/den group sits behind no-sync hints on all 4 P
            # producers so the col-strip matmuls stay adjacent on PE and run
            # concurrently.
            for kc in range(KC):
                hf = kc // 2
                group_deps = [p_prods[(h, hf)] for h in range(HEADS)]
                for h in range(HEADS):
                    p = p_tiles[(h, hf)]
                    psl = p[:, (kc % 2) * 512 : (kc % 2 + 1) * 512]
                    mm1 = nc.tensor.matmul(
                        ot_ps[32 * h : 32 * h + 32, :],
                        lhsT=v_sb[:, kc * 128 + 32 * h : kc * 128 + 32 * h + 32],
                        rhs=psl,
                        start=False,
                        stop=(kc == KC - 1),
                        tile_position=(0, 32 * h),
                        skip_group_check=True,
                    )
                    mm2 = nc.tensor.matmul(
                        d_ps[32 * h : 32 * h + 32, :],
                        lhsT=ones_sb[:],
                        rhs=psl,
                        start=False,
                        stop=(kc == KC - 1),
                        tile_position=(0, 32 * h),
                        skip_group_check=True,
                    )
                    for dep in group_deps:
                        tile.add_dep_helper(mm1.ins, dep, False, "pv pack")
                        tile.add_dep_helper(mm2.ins, dep, False, "pv pack")

            # d_ps rows 32h..32h+31 all hold head h's denominator (the ones
            # lhsT replicates it), so 1/d_ps IS the broadcast divisor.
            # 18-bit approx is plenty: den ~ 512 +- 15%.
            invden = misc.tile([128, N], FP32, tag="invden")
            nc.vector.reciprocal_approx_fast(invden[:], d_ps[:])
            otn = misc.tile([128, N], BF16, tag="otn")
            nc.vector.tensor_mul(otn[:], ot_ps[:], invden[:])

            # transposed proj: outT[c, q] = sum_hd projwT[hd, c] * otn[hd, q];
            # the PSUM->SBUF copy is mandatory before DMA, so the bias add
            # rides it for free
            pr_ps = mm_ps.tile([128, N], FP32, tag="ps2")
            nc.tensor.matmul(pr_ps[:], lhsT=pw_sb[:], rhs=otn[:], start=True, stop=True)
            ot = outp.tile([128, N], FP32, tag="out")
            nc.vector.tensor_add(ot[:], pr_ps[:], pb_sb[:])
            nc.sync.dma_start(out_d[b], ot[:])
    nc.compile()
    return nc


def TileCtx(nc):
    return tile.TileContext(nc)


_CACHE = {}


def _get_program():
    if "nc" not in _CACHE:
        _CACHE["nc"] = _build_program()
    return _CACHE["nc"]


def _host_prep(x, y, H, W, D, qkv_w, qkv_b, proj_w, proj_b,
               pos_proj_w, pos_proj_b, ln1_g, ln1_b, p1_w, p1_b,
               ln2_g, ln2_b, p2_w, p2_b, ln3_g, ln3_b, p3_w, p3_b):
    """Numpy-only prep: layout transforms, weight folding, pos-bias tables."""
    scale = HD ** -0.5
    bf = ml_dtypes.bfloat16

    xT = np.ascontiguousarray(x.transpose(0, 2, 1)).astype(bf)  # (B_, C, N)
    yT = np.ascontiguousarray(y.transpose(0, 2, 1)).astype(bf)

    wqT = np.ascontiguousarray((qkv_w[0:C] * scale).T).astype(bf)
    wkT = np.ascontiguousarray(qkv_w[C : 2 * C].T).astype(bf)
    wvT = np.ascontiguousarray(qkv_w[2 * C : 3 * C].T).astype(bf)
    projwT = np.ascontiguousarray(proj_w.T).astype(bf)

    # pos-bias MLP (tiny: 3375x8), exact fp32 replica of the reference math
    biases, idx = _rel_pos_tables(int(H), int(W), int(D))
    pos = biases @ pos_proj_w.T + pos_proj_b
    pos = np.maximum(_layernorm(pos, ln1_g, ln1_b), 0) @ p1_w.T + p1_b
    pos = np.maximum(_layernorm(pos, ln2_g, ln2_b), 0) @ p2_w.T + p2_b
    pos = np.maximum(_layernorm(pos, ln3_g, ln3_b), 0) @ p3_w.T + p3_b  # (T, h)
    rpb = pos[idx.reshape(-1)].reshape(N, N, HEADS)  # [q, k, h]
    bq = qkv_b[0:C]
    bk = qkv_b[C : 2 * C]
    if np.any(bq) or np.any(bk):
        raise NotImplementedError("nonzero qkv bias not supported")
    rpbT = rpb.transpose(2, 1, 0)  # [h, k, q]
    # layout (h', 128, KC*N): partition p = k%128, col kc*N+q
    rpbT = rpbT.reshape(HEADS, KC, 128, N).transpose(0, 2, 1, 3).reshape(HEADS, 128, KC * N)
    rT = np.ascontiguousarray(
        np.concatenate([rpbT[h] for h in PE_HEADS], axis=1)
    ).astype(bf)
    erT = np.ascontiguousarray(
        np.concatenate([np.exp(rpbT[h]) for h in DVE_HEADS], axis=1)
    ).astype(bf)

    ident = np.eye(128, dtype=np.float32).astype(bf)

    pb_full = proj_b + qkv_b[2 * C : 3 * C] @ proj_w.T  # fold v bias thru proj
    pbCN = np.tile(pb_full[:, None], (1, N)).astype(np.float32)  # (C, N)

    return xT, yT, rT, erT, ident, wqT, wkT, wvT, projwT, pbCN


def kernel(**inputs):
    inputs = {k: np.asarray(v) if not np.isscalar(v) else v for k, v in inputs.items()}
    x = np.asarray(inputs["x"], np.float32)
    assert x.shape == (B_, N, C)
    xT, yT, rT, erT, ident, wqT, wkT, wvT, projwT, pbCN = _host_prep(
        np.asarray(inputs["x"], np.float32),
        np.asarray(inputs["y"], np.float32),
        inputs["H"], inputs["W"], inputs["D"],
        np.asarray(inputs["qkv_w"], np.float32),
        np.asarray(inputs["qkv_b"], np.float32),
        np.asarray(inputs["proj_w"], np.float32),
        np.asarray(inputs["proj_b"], np.float32),
        np.asarray(inputs["pos_proj_w"], np.float32),
        np.asarray(inputs["pos_proj_b"], np.float32),
        np.asarray(inputs["ln1_g"], np.float32), np.asarray(inputs["ln1_b"], np.float32),
        np.asarray(inputs["p1_w"], np.float32), np.asarray(inputs["p1_b"], np.float32),
        np.asarray(inputs["ln2_g"], np.float32), np.asarray(inputs["ln2_b"], np.float32),
        np.asarray(inputs["p2_w"], np.float32), np.asarray(inputs["p2_b"], np.float32),
        np.asarray(inputs["ln3_g"], np.float32), np.asarray(inputs["ln3_b"], np.float32),
        np.asarray(inputs["p3_w"], np.float32), np.asarray(inputs["p3_b"], np.float32),
    )

    nc = _get_program()
    in_maps = []
    for c in range(NCORES):
        sl = slice(c * WIN, (c + 1) * WIN)
        in_maps.append(
            {
                "xT": xT[sl],
                "yT": yT[sl],
                "rT": rT,
                "expRT": erT,
                "ident": ident,
                "wqT": wqT,
                "wkT": wkT,
                "wvT": wvT,
                "projwT": projwT,
                "pbCN": pbCN,
            }
        )
    kwargs = {}
    if PROFILE:
        kwargs = dict(trace=True, **PROFILE_KWARGS)
    res = bass_utils.run_bass_kernel_spmd(
        nc, in_maps, core_ids=list(range(NCORES)), **kwargs
    )
    global LAST_EXEC_NS, LAST_RESULTS
    LAST_EXEC_NS = res.exec_time_ns
    LAST_RESULTS = res
    # outT is (WIN, C, N); un-transpose to (WIN, N, C) on host
    out = np.concatenate(
        [np.asarray(r["outT"]).transpose(0, 2, 1) for r in res.results], axis=0
    )
    return np.ascontiguousarray(out).astype(np.float32)


PROFILE = False
PROFILE_KWARGS = {}
LAST_EXEC_NS = None
LAST_RESULTS = None


if __name__ == "__main__":
    # smoke test with random data
    rng = np.random.default_rng(0)
    demo = {
        "x": rng.standard_normal((B_, N, C)).astype(np.float32),
        "y": rng.standard_normal((B_, N, C)).astype(np.float32),
        "H": 8, "W": 8, "D": 8,
        "qkv_w": (rng.standard_normal((3 * C, C)) * 0.02).astype(np.float32),
        "qkv_b": np.zeros(3 * C, np.float32),
        "proj_w": (rng.standard_normal((C, C)) * 0.02).astype(np.float32),
        "proj_b": np.zeros(C, np.float32),
        "pos_proj_w": (rng.standard_normal((POS_DIM, 3)) * 0.02).astype(np.float32),
        "pos_proj_b": np.zeros(POS_DIM, np.float32),
        "ln1_g": np.ones(POS_DIM, np.float32), "ln1_b": np.zeros(POS_DIM, np.float32),
        "p1_w": (rng.standard_normal((POS_DIM, POS_DIM)) * 0.02).astype(np.float32),
        "p1_b": np.zeros(POS_DIM, np.float32),
        "ln2_g": np.ones(POS_DIM, np.float32), "ln2_b": np.zeros(POS_DIM, np.float32),
        "p2_w": (rng.standard_normal((POS_DIM, POS_DIM)) * 0.02).astype(np.float32),
        "p2_b": np.zeros(POS_DIM, np.float32),
        "ln3_g": np.ones(POS_DIM, np.float32), "ln3_b": np.zeros(POS_DIM, np.float32),
        "p3_w": (rng.standard_normal((HEADS, POS_DIM)) * 0.02).astype(np.float32),
        "p3_b": np.zeros(HEADS, np.float32),
    }
    out = kernel(**demo)
    print("kernel out:", out.shape, out.dtype, np.abs(out).max())
